# revision 36
# baseline (speedup 1.0000x reference)
"""Bottleneck-MHSA fused kernel for 8 Trainium2 NeuronCores.

Sharding: core c = 2*b + s handles batch b; attention queries are split in
half between the two cores of a pair. Each core computes conv1 + BN1 + qkv
for its whole batch (redundantly with its pair partner), then attention for
all 4 heads over its query half, then BN2 + W3 + BN3 + residual for its
query half. Cross-core traffic is only the three tiny BN statistics
AllReduces.

Key optimizations over the straightforward version:
- bv and bk are dropped: BN2 absorbs per-channel shifts (bv), and bk only
  adds a per-query constant to scores, which softmax cancels.
- conv1 stats (sum/sumsq) are fused into the PSUM->SBUF drain on the
  Scalar engine via activation accum_out; DVE does no work in conv1.
- BN applies are single Scalar activation ops Relu(scale*x+bias).
- rsqrt via DVE Newton iteration, so the Scalar act table never swaps
  (exp/copy/relu/square/identity live in one table).
- softmax denominator reciprocal runs on 64 partitions (broadcast first).
- BN3 stats come from the Gram matrix W3 (out2 out2^T) W3^T diagonal,
  computed *before* the W3 matmul so the last collective overlaps it and
  the finale drains PSUM directly (no y3 staging).
"""
import numpy as np

HEADS = 4
DH = 64
C = 256          # PLANES
CIN = 1024       # IN_PLANES
N = 2744         # tokens per batch
NQ = N // 2      # query half per core
B = 4
EPS = 1e-5
N_CORES = 8
CNT12 = 8 * N    # BN1 effective count (pairs double-count; mean/var exact)
CNT2 = 8 * NQ    # BN2/BN3 count (distinct shards)


def _chunks(total, n):
    # even sizes (fp32r matmul requires an even moving free dim)
    assert total % 2 == 0
    half = total // 2
    sizes = [2 * (half // n + (1 if i < half % n else 0)) for i in range(n)]
    out, off = [], 0
    for s in sizes:
        out.append((off, s))
        off += s
    return out


CH6 = _chunks(N, 6)     # key/token chunks (456/458 wide, all >=256 for f32r)
CH3 = _chunks(NQ, 3)    # query chunks
MT22 = [(t * 128, min(128, N - t * 128)) for t in range((N + 127) // 128)]
TQ11 = [(t * 128, min(128, NQ - t * 128)) for t in range((NQ + 127) // 128)]


def build_program():
    from concourse import bacc, mybir, tile

    F32 = mybir.dt.float32
    F32R = mybir.dt.float32r
    I32 = mybir.dt.int32
    BF16 = mybir.dt.bfloat16

    nc = bacc.Bacc("TRN2", target_bir_lowering=False, debug=False,
                   num_devices=N_CORES)

    # ---- I/O ----
    io = {}
    io["X"] = nc.dram_tensor("X", [CIN, N], F32R, kind="ExternalInput").ap()
    io["W1T"] = nc.dram_tensor("W1T", [CIN, C], F32R, kind="ExternalInput").ap()
    io["WQT"] = nc.dram_tensor("WQT", [C, C], F32R, kind="ExternalInput").ap()
    io["WVT"] = nc.dram_tensor("WVT", [C, C], F32R, kind="ExternalInput").ap()
    io["W3T"] = nc.dram_tensor("W3T", [C, CIN], F32R, kind="ExternalInput").ap()
    io["W3N"] = nc.dram_tensor("W3N", [CIN, C], F32, kind="ExternalInput").ap()
    io["WKQ"] = nc.dram_tensor("WKQ", [HEADS, C, 128], F32R, kind="ExternalInput").ap()
    io["REL"] = nc.dram_tensor("REL", [HEADS, DH, NQ], F32R, kind="ExternalInput").ap()
    io["BQ"] = nc.dram_tensor("BQ", [HEADS, DH], F32, kind="ExternalInput").ap()
    io["IDT"] = nc.dram_tensor("IDT", [128, 128], F32R, kind="ExternalInput").ap()
    io["GB1"] = nc.dram_tensor("GB1", [2, C], F32, kind="ExternalInput").ap()
    io["GB2"] = nc.dram_tensor("GB2", [2, C], F32, kind="ExternalInput").ap()
    io["GB3"] = nc.dram_tensor("GB3", [2, CIN], F32, kind="ExternalInput").ap()
    io["OUT"] = nc.dram_tensor("OUT", [CIN, NQ], F32, kind="ExternalOutput").ap()

    with tile.TileContext(nc) as tc:
        _emit(nc, tc, mybir, F32, F32R, I32, BF16, io)

    nc.compile()
    from concourse.bass_interp import get_hw_module
    nc.m = get_hw_module(nc.m)
    return nc


def _emit(nc, tc, mybir, F32, F32R, I32, BF16, io):
    import contextlib

    AX = mybir.AluOpType
    AF = mybir.ActivationFunctionType
    X_AXIS = mybir.AxisListType.X

    Xd, W1T, WQT, WVT, W3T = io["X"], io["W1T"], io["WQT"], io["WVT"], io["W3T"]
    W3N, WKQ, RELd, BQd, IDTd = io["W3N"], io["WKQ"], io["REL"], io["BQ"], io["IDT"]
    GB1, GB2, GB3, OUTd = io["GB1"], io["GB2"], io["GB3"], io["OUT"]

    def stats_collective(src_sbuf, width, out_gst, tag):
        """AllReduce [P, width] partials over all 8 cores."""
        p = src_sbuf.shape[0]
        cin = dpool.tile([p, width], F32, name=f"ccin_{tag}")
        cout = dpool.tile([p, width], F32, addr_space="Shared",
                          name=f"ccout_{tag}")
        nc.sync.dma_start(cin[:], src_sbuf[:])
        nc.gpsimd.collective_compute(
            "AllReduce", AX.add,
            replica_groups=[list(range(N_CORES))],
            ins=[cin.opt()], outs=[cout.opt()],
        )
        nc.sync.dma_start(out_gst[:], cout[:])

    def rsqrt_newton(y, x, tag):
        """y = 1/sqrt(x) on DVE only (bit-trick seed + 2 Newton steps)."""
        p, w = x.shape[0], x.shape[1]
        xi = x[:].bitcast(I32)
        t1 = wpool.tile([p, w], I32, name=f"rsq_t1_{tag}")
        nc.vector.tensor_scalar(t1[:], xi, 1, None, AX.arith_shift_right)
        yi = y[:].bitcast(I32)
        nc.vector.tensor_scalar(yi, t1[:], -1, 0x5f3759df, AX.mult, AX.add)
        h = wpool.tile([p, w], F32, name=f"rsq_h_{tag}")
        for _ in range(2):
            nc.vector.tensor_tensor(h[:], y[:], y[:], AX.mult)
            nc.vector.tensor_tensor(h[:], x[:], h[:], AX.mult)
            nc.vector.tensor_scalar(h[:], h[:], -0.5, 1.5, AX.mult, AX.add)
            nc.vector.tensor_tensor(y[:], y[:], h[:], AX.mult)

    def bn_coeffs(tot, gt, bt, cnt, w, sc, cc, tag):
        """tot [P, 2w] = [sums | sumsqs] -> scale sc [P, w], bias cc [P, w]."""
        p = tot.shape[0]
        mean = wpool.tile([p, w], F32, name=f"mean_{tag}")
        var = wpool.tile([p, w], F32, name=f"var_{tag}")
        nc.vector.tensor_scalar_mul(mean[:], tot[:, 0:w], 1.0 / cnt)
        nc.vector.tensor_scalar_mul(var[:], tot[:, w:2 * w], 1.0 / cnt)
        m2 = wpool.tile([p, w], F32, name=f"m2_{tag}")
        nc.vector.tensor_tensor(m2[:], mean[:], mean[:], AX.mult)
        nc.vector.tensor_tensor(var[:], var[:], m2[:], AX.subtract)
        nc.vector.tensor_scalar_add(var[:], var[:], EPS)
        rstd = wpool.tile([p, w], F32, name=f"rstd_{tag}")
        rsqrt_newton(rstd, var, tag)
        nc.vector.tensor_tensor(sc[:], gt[:], rstd[:], AX.mult)
        tmp = wpool.tile([p, w], F32, name=f"tmpc_{tag}")
        nc.vector.tensor_tensor(tmp[:], sc[:], mean[:], AX.mult)
        nc.vector.tensor_tensor(cc[:], bt[:], tmp[:], AX.subtract)

    with contextlib.ExitStack() as top:
        wpool = top.enter_context(tc.tile_pool(name="wpool", bufs=1))
        dpool = top.enter_context(tc.tile_pool(name="dpool", bufs=1, space="DRAM"))

        # f32r constants (memset to f32r is rejected at codegen; go via f32)
        onespad = wpool.tile([128, 128], F32, name="onespad")
        nc.vector.memset(onespad[:], 1.0)

        # ---- startup barrier, as early as possible: the first collective
        # pays ~66us of CC firmware warmup per core, so trigger it
        # immediately and let it warm up under conv1 ----
        bar_in = dpool.tile([1, 2], F32, name="bar_in")
        bar_out = dpool.tile([1, 2], F32, addr_space="Shared", name="bar_out")
        nc.gpsimd.dma_start(bar_in[:], onespad[0:1, 0:2])
        nc.gpsimd.collective_compute(
            "AllReduce", AX.add,
            replica_groups=[list(range(N_CORES))],
            ins=[bar_in.opt()], outs=[bar_out.opt()],
        )

        onesr = wpool.tile([128, DH], F32R, name="onesr")
        nc.vector.tensor_copy(onesr[:], onespad[:, 0:DH])

        # ---- phase-1 weights only; the rest loads after the conv loop so X
        # gets the DMA bandwidth ----
        w1t = []
        for k in range(8):
            t = wpool.tile([128, C], F32R, name=f"w1t{k}")
            nc.scalar.dma_start(t[:], W1T[k * 128:(k + 1) * 128, :])
            w1t.append(t)

        def warm_pe(n, tag):
            # keep the PE busy through idle windows: the clock ramps to max
            # only after ~3us of sustained work, and idle gaps reset it
            with tc.tile_pool(name=f"warm_{tag}", bufs=1, space="PSUM") as wp:
                pw = wp.tile([128, 256], F32, name=f"pw_{tag}")
                for _ in range(n):
                    nc.tensor.matmul(pw[:], w1t[0][:, 0:128], w1t[0][:, 0:256],
                                     start=True, stop=True)
                sink = wpool.tile([1, 2], F32, name=f"wsink_{tag}")
                nc.scalar.activation(sink[:], pw[0:1, 0:2], AF.Copy)

        warm_pe(20, "a")
        g1t = wpool.tile([128, 2], F32, name="g1t")
        b1t = wpool.tile([128, 2], F32, name="b1t")
        nc.scalar.dma_start(g1t[:], GB1[0].rearrange("(m p) -> p m", p=128))
        nc.scalar.dma_start(b1t[:], GB1[1].rearrange("(m p) -> p m", p=128))

        # stats accumulators
        S1 = wpool.tile([128, 12], F32, name="S1")   # conv1 sums   (mt*6+ci)
        Q1 = wpool.tile([128, 12], F32, name="Q1")   # conv1 sumsqs
        S2 = wpool.tile([DH, 16], F32, name="S2")    # attn sums    (h*4+ci)
        nc.vector.memset(S2[:], 0.0)
        Q2 = wpool.tile([DH, 16], F32, name="Q2")
        nc.vector.memset(Q2[:], 0.0)

        OUT2 = [wpool.tile([128, NQ], F32R, name=f"out2_{m}") for m in range(2)]

        with contextlib.ExitStack() as ph_a:
            qpool = ph_a.enter_context(tc.tile_pool(name="qpool", bufs=1))
            KHAT = [qpool.tile([128, N], F32R, name=f"khat{h}") for h in range(HEADS)]
            QHAT = [qpool.tile([128, NQ], F32R, name=f"qhat{h}") for h in range(HEADS)]
            # one VTON tile for all heads: head h block = cols [h*1430, h*1430+1430)
            # laid out as 22 tiles of [v(64) | ones(1)]; bf16 is enough for
            # softmax-side values and halves the copy cost
            VTON = qpool.tile([128, HEADS * 22 * 65], BF16, name="vton")
            vt4 = VTON[:].rearrange("p (h t c) -> p h t c", h=HEADS, c=65)
            # ones columns for all heads/tiles in one strided write
            nc.vector.tensor_copy(
                vt4[:, :, :, 64],
                onespad[:, 0:HEADS * 22].rearrange("p (h t) -> p h t", h=HEADS))
            for h in range(HEADS):
                nc.scalar.dma_start(QHAT[h][DH:128, :], RELd[h])

            with contextlib.ExitStack() as ph1:
                y1pool = ph1.enter_context(tc.tile_pool(name="y1pool", bufs=1))
                Y1 = [y1pool.tile([128, N], F32R, name=f"y1_{m}") for m in range(2)]

                # ---- phase 1: conv1 (y1 = W1 @ x), stats fused into the
                # Scalar-engine PSUM drain (copy+accum, square+accum).
                # X loads in 914-wide tiles: >=3.6KB per partition line keeps
                # the DMA engines at full rate ----
                XCH = [(0, 914), (914, 914), (1828, 916)]
                with tc.tile_pool(name="xpool", bufs=2) as xpool, \
                     tc.tile_pool(name="psum1", bufs=3, space="PSUM") as psum1:
                    # software-pipelined X prefetch over 3 DMA queues
                    # (~85GB/s per queue): issue groups 0-1 up front, group 2
                    # after group 0's compute so its triggers never block the
                    # Scalar-queue drains (bufs=2 WAR)
                    def xfetch(c3):
                        xoff, xsz = XCH[c3]
                        row = []
                        for k in range(8):
                            t = xpool.tile([128, xsz], F32R, name=f"xc{k}",
                                           tag=f"xc{k}")
                            eng = (nc.sync, nc.gpsimd, nc.scalar)[k % 3]
                            eng.dma_start(t[:], Xd[k * 128:(k + 1) * 128,
                                                   xoff:xoff + xsz])
                            row.append(t)
                        return row
                    xtiles = [xfetch(0), xfetch(1), None]
                    for c3, (xoff, xsz) in enumerate(XCH):
                        xts = xtiles[c3]
                        # sub-chunk offsets must be even (f32r alignment)
                        halves = [(0, xsz // 2 - (xsz // 2) % 2)]
                        halves.append((halves[0][1], xsz - halves[0][1]))
                        for half, (hoff, hsz) in enumerate(halves):
                            for mt in range(2):
                                ps = psum1.tile([128, hsz], F32, name="pconv",
                                                tag="pconv", padded_shape=[128, 458])
                                for k in range(8):
                                    nc.tensor.matmul(
                                        ps[:], w1t[k][:, mt * 128:(mt + 1) * 128],
                                        xts[k][:, hoff:hoff + hsz],
                                        start=(k == 0), stop=(k == 7))
                                off = xoff + hoff
                                idx = mt * 6 + (c3 * 2 + half)
                                nc.scalar.activation(Y1[mt][:, off:off + hsz], ps[:],
                                                     AF.Copy,
                                                     accum_out=S1[:, idx:idx + 1])
                                sq = xpool.tile([128, hsz], F32, name="sqs",
                                                tag="sqs", padded_shape=[128, 458])
                                nc.scalar.activation(sq[:], ps[:], AF.Square,
                                                     accum_out=Q1[:, idx:idx + 1])
                        if c3 == 0:
                            xtiles[2] = xfetch(2)

                # ---- remaining weights (deferred so X had DMA priority) ----
                wqt, wvt = [], []
                for srcw, dst, nm in ((WQT, wqt, "wq"), (WVT, wvt, "wv")):
                    for k in range(2):
                        t = wpool.tile([128, C], F32R, name=f"{nm}{k}")
                        nc.scalar.dma_start(t[:], srcw[k * 128:(k + 1) * 128, :])
                        dst.append(t)
                wkqt = []
                for h in range(HEADS):
                    row = []
                    for k in range(2):
                        t = wpool.tile([128, 128], F32R, name=f"wkq{h}_{k}")
                        nc.gpsimd.dma_start(t[:], WKQ[h][k * 128:(k + 1) * 128, :])
                        row.append(t)
                    wkqt.append(row)
                bqt = wpool.tile([DH, HEADS], F32, name="bqt")
                nc.scalar.dma_start(bqt[:], BQd[:].rearrange("h d -> d h"))
                g2t = wpool.tile([DH, HEADS], F32, name="g2t")
                b2t = wpool.tile([DH, HEADS], F32, name="b2t")
                nc.scalar.dma_start(g2t[:], GB2[0].rearrange("(h d) -> d h", d=DH))
                nc.scalar.dma_start(b2t[:], GB2[1].rearrange("(h d) -> d h", d=DH))
                w3t = []
                for k in range(2):
                    t = wpool.tile([128, CIN], F32R, name=f"w3t{k}")
                    nc.sync.dma_start(t[:], W3T[k * 128:(k + 1) * 128, :])
                    w3t.append(t)
                w3n = []
                for mt in range(8):
                    t = wpool.tile([128, C], F32, name=f"w3n{mt}")
                    nc.sync.dma_start(t[:], W3N[mt * 128:(mt + 1) * 128, :])
                    w3n.append(t)
                idt = wpool.tile([128, 128], F32R, name="idt")
                nc.sync.dma_start(idt[:], IDTd[:])
                g3t = wpool.tile([128, 8], F32, name="g3t")
                b3t = wpool.tile([128, 8], F32, name="b3t")
                nc.sync.dma_start(g3t[:], GB3[0].rearrange("(m p) -> p m", p=128))
                nc.sync.dma_start(b3t[:], GB3[1].rearrange("(m p) -> p m", p=128))

                # ---- phase 1b: BN1 stats collective + coeffs ----
                s1sum = wpool.tile([128, 2], F32, name="s1sum")
                q1sum = wpool.tile([128, 2], F32, name="q1sum")
                nc.vector.reduce_sum(s1sum[:], S1[:].rearrange("p (m c) -> p m c", c=6), X_AXIS)
                nc.vector.reduce_sum(q1sum[:], Q1[:].rearrange("p (m c) -> p m c", c=6), X_AXIS)
                st1 = wpool.tile([128, 4], F32, name="st1")
                nc.vector.tensor_copy(st1[:, 0:2], s1sum[:])
                nc.vector.tensor_copy(st1[:, 2:4], q1sum[:])
                tot1 = wpool.tile([128, 4], F32, name="tot1")
                stats_collective(st1, 4, tot1, "bn1")
                s1c = wpool.tile([128, 2], F32, name="s1c")
                c1c = wpool.tile([128, 2], F32, name="c1c")
                bn_coeffs(tot1, g1t, b1t, CNT12, 2, s1c, c1c, "bn1")

                warm_pe(24, "b")

                # ---- phase 2: out1 = relu(s*y1 + c), in place, Scalar ----
                OUT1 = [Y1[m][:] for m in range(2)]
                for (off, sz) in CH6:
                    for mt in range(2):
                        nc.scalar.activation(OUT1[mt][:, off:off + sz],
                                             Y1[mt][:, off:off + sz],
                                             AF.Relu,
                                             bias=c1c[:, mt:mt + 1],
                                             scale=s1c[:, mt:mt + 1])

                # ---- phase 3a: vT = out1^T @ WvT (no bias; BN2 absorbs bv) ----
                with tc.tile_pool(name="psum3a", bufs=3, space="PSUM") as psum3a:
                    for t, (mo, msz) in enumerate(MT22):
                        ps = psum3a.tile([128, C], F32, name="pvt", tag="pvt")
                        for k in range(2):
                            nc.tensor.matmul(ps[0:msz, :], OUT1[k][:, mo:mo + msz],
                                             wvt[k][:], start=(k == 0), stop=(k == 1))
                        nc.vector.tensor_copy(
                            vt4[0:msz][:, :, t, 0:64],
                            ps[0:msz, :].rearrange("p (h d) -> p h d", h=HEADS))

                # ---- phase 3b: KHAT = [k; q] raw (biases cancel in softmax),
                # QHAT q-half with bq ----
                with tc.tile_pool(name="psum3b", bufs=3, space="PSUM") as psum3b:
                    for h in range(HEADS):
                        hs = h * DH
                        for (off, sz) in CH6:
                            ps = psum3b.tile([128, sz], F32, name="pkh", tag="pkh")
                            for k in range(2):
                                nc.tensor.matmul(ps[:], wkqt[h][k][:],
                                                 OUT1[k][:, off:off + sz],
                                                 start=(k == 0), stop=(k == 1))
                            nc.scalar.activation(KHAT[h][:, off:off + sz], ps[:],
                                                 AF.Copy)
                        for (off, sz) in CH3:
                            pq = psum3b.tile([DH, sz], F32, name="pqh", tag="pqh")
                            for k in range(2):
                                nc.tensor.matmul(pq[:], wqt[k][:, hs:hs + DH],
                                                 OUT1[k][:, off:off + sz],
                                                 start=(k == 0), stop=(k == 1))
                            nc.scalar.activation(QHAT[h][0:DH, off:off + sz], pq[:],
                                                 AF.Identity, bias=bqt[:, h:h + 1])

            # ---- phase 4: attention (S^T layout, staged exp, fused denom) ----
            with tc.tile_pool(name="oattp", bufs=1) as oattp, \
                 tc.tile_pool(name="epool", bufs=2) as epool, \
                 tc.tile_pool(name="psum4", bufs=1, space="PSUM") as psum4:
                OATT = [oattp.tile([DH, NQ], F32R, name=f"oatt{h}") for h in range(HEADS)]
                QP = [(0, 1024, [(0, 512), (512, 512)]),
                      (1024, 348, [(0, 348)])]
                for h in range(HEADS):
                    for qo, qw, subs in QP:
                        # pav0 double-buffered so the next group's AV matmuls
                        # don't wait on this group's softmax-denominator drain
                        pavs = [psum4.tile([65, sz], F32, name=f"pav{si}",
                                           tag=f"pav{si}", bufs=(2 if si == 0 else 1))
                                for si, (so, sz) in enumerate(subs)]
                        for t, (mo, msz) in enumerate(MT22):
                            ps = psum4.tile([128, qw], F32, name="ps", tag="ps", bufs=2,
                                            padded_shape=[128, 1024])
                            for so, sz in subs:
                                nc.tensor.matmul(ps[0:msz, so:so + sz],
                                                 KHAT[h][:, mo:mo + msz],
                                                 QHAT[h][:, qo + so:qo + so + sz],
                                                 start=True, stop=True)
                            e = epool.tile([128, qw], BF16, name="e", tag="e", bufs=6)
                            nc.scalar.activation(e[0:msz, :], ps[0:msz, :], AF.Exp)
                            for si, (so, sz) in enumerate(subs):
                                nc.tensor.matmul(pavs[si][:],
                                                 VTON[0:msz, h * 1430 + 65 * t:
                                                      h * 1430 + 65 * t + 65],
                                                 e[0:msz, so:so + sz],
                                                 start=(t == 0), stop=(t == 21))
                        for si, (so, sz) in enumerate(subs):
                            pav = pavs[si]
                            off = qo + so
                            # denominator row -> SBUF (Scalar), broadcast to 64
                            # partitions (PE), then reciprocal on 64 lanes (DVE)
                            # on DVE: the Scalar queue is backed up with EXPs,
                            # which would stall the pb matmul ~3.5us
                            den = epool.tile([65, sz], F32R, name="den", tag="den",
                                             bufs=2)
                            nc.vector.tensor_copy(den[DH:65, :], pav[DH:65, :])
                            pb = psum4.tile([DH, sz], F32, name="pb", tag="pb", bufs=1)
                            nc.tensor.matmul(pb[:], onesr[DH:65, :], den[DH:65, :],
                                             start=True, stop=True)
                            pbs = epool.tile([DH, sz], F32R, name="pbs", tag="pbs", bufs=2)
                            with nc.allow_low_precision(reason="softmax denom recip"):
                                nc.vector.reciprocal(pbs[:], pb[:])
                            nc.vector.tensor_tensor(OATT[h][:, off:off + sz],
                                                    pav[0:DH, :], pbs[:], AX.mult)
                            idx = h * 4 + (0 if qo == 0 else 2) + si
                            nc.vector.reduce_sum(S2[:, idx:idx + 1],
                                                 OATT[h][:, off:off + sz], X_AXIS)
                            sq2 = epool.tile([DH, sz], F32, name="sq2", tag="sq2", bufs=2)
                            nc.vector.tensor_tensor(sq2[:], OATT[h][:, off:off + sz],
                                                    OATT[h][:, off:off + sz], AX.mult)
                            nc.vector.reduce_sum(Q2[:, idx:idx + 1], sq2[:], X_AXIS)

                # ---- phase 5: BN2 + relu (Scalar), assemble OUT2 ----
                s2sum = wpool.tile([DH, 4], F32, name="s2sum")
                q2sum = wpool.tile([DH, 4], F32, name="q2sum")
                nc.vector.reduce_sum(s2sum[:], S2[:].rearrange("p (h c) -> p h c", c=4), X_AXIS)
                nc.vector.reduce_sum(q2sum[:], Q2[:].rearrange("p (h c) -> p h c", c=4), X_AXIS)
                st2 = wpool.tile([DH, 8], F32, name="st2")
                nc.vector.tensor_copy(st2[:, 0:4], s2sum[:])
                nc.vector.tensor_copy(st2[:, 4:8], q2sum[:])
                tot2 = wpool.tile([DH, 8], F32, name="tot2")
                stats_collective(st2, 8, tot2, "bn2")
                s2c = wpool.tile([DH, 4], F32, name="s2c")
                c2c = wpool.tile([DH, 4], F32, name="c2c")
                bn_coeffs(tot2, g2t, b2t, CNT2, 4, s2c, c2c, "bn2")
                for h in range(HEADS):
                    nc.scalar.activation(OATT[h][:], OATT[h][:], AF.Relu,
                                         bias=c2c[:, h:h + 1],
                                         scale=s2c[:, h:h + 1])
                    nc.scalar.dma_start(
                        OUT2[h // 2][(h % 2) * DH:(h % 2) * DH + DH, :], OATT[h][:])

        # ---- phase 6: BN3 stats via Gram (before W3!), then W3 + finale ----
        with tc.tile_pool(name="otpool", bufs=1) as otpool, \
             tc.tile_pool(name="fpool", bufs=2) as fpool:
            XR = []
            for mt in range(8):
                xr = fpool.tile([128, NQ], F32, name=f"xr{mt}", tag=f"xr{mt}", bufs=1)
                eng = (nc.sync, nc.gpsimd, nc.scalar)[mt % 3]
                eng.dma_start(xr[:], Xd[mt * 128:(mt + 1) * 128, 0:NQ].bitcast(F32))
                XR.append(xr)

            warm_pe(20, "c")

            # out2^T chunks [tok, 256] via PE transpose
            OT = [otpool.tile([128, C], F32R, name=f"ot{t}") for t in range(11)]
            with tc.tile_pool(name="psumT", bufs=4, space="PSUM") as psumT:
                for t, (qo, qsz) in enumerate(TQ11):
                    for mt in range(2):
                        pt = psumT.tile([qsz, 128], F32R, name="pt", tag="pt",
                                        bufs=2, padded_shape=[128, 128])
                        nc.tensor.transpose(pt[:], OUT2[mt][:, qo:qo + qsz],
                                            idt[:])
                        nc.scalar.activation(OT[t][0:qsz, mt * 128:(mt + 1) * 128],
                                             pt[:], AF.Copy)
                # token sums of out2 for the y3 channel sums; zero-padded
                # columns keep the matvec free dim even (f32r requirement)
                sv = wpool.tile([128, 4], F32R, name="sv")
                nc.vector.memset(sv[:].bitcast(F32), 0.0)
                with nc.allow_low_precision(reason="f32r view of f32 sums"):
                    for mt in range(2):
                        nc.vector.reduce_sum(sv[:, 2 * mt:2 * mt + 1],
                                             OUT2[mt][:], X_AXIS)
                # Gram G = out2 @ out2^T, [256,256] as 2 x [128,256]
                Gsb = [wpool.tile([128, C], F32R, name=f"gsb{kb}") for kb in range(2)]
                for kb in range(2):
                    pg = psumT.tile([128, C], F32, name="pg", tag="pg", bufs=1)
                    for t, (qo, qsz) in enumerate(TQ11):
                        nc.tensor.matmul(pg[:], OT[t][0:qsz, kb * 128:(kb + 1) * 128],
                                         OT[t][0:qsz, :],
                                         start=(t == 0), stop=(t == 10))
                    nc.scalar.activation(Gsb[kb][:], pg[:], AF.Copy)

            st3 = wpool.tile([128, 16], F32, name="st3")
            with tc.tile_pool(name="psumG", bufs=2, space="PSUM") as psumG:
                # sums: y3_sum[o] = W3 @ sum_n(out2), computed directly in
                # [128, 8] channel layout (out partition = channel-in-block)
                pstats = psumG.tile([128, 16], F32, name="pstats", tag="pstats",
                                    bufs=1)
                for mt in range(8):
                    for kb in range(2):
                        nc.tensor.matmul(pstats[:, 2 * mt:2 * mt + 2],
                                         w3t[kb][:, mt * 128:(mt + 1) * 128],
                                         sv[:, 2 * kb:2 * kb + 2],
                                         start=(kb == 0), stop=(kb == 1))
                nc.scalar.activation(
                    st3[:, 0:8],
                    pstats[:].rearrange("p (c two) -> p c two", two=2)[:, :, 0],
                    AF.Copy)
                # sumsqs: diag(W3 G W3^T) = rowsum((W3 @ G) * W3)
                for mt in range(8):
                    pt1 = psumG.tile([128, C], F32, name="pt1", tag="pt1", bufs=2)
                    for kb in range(2):
                        nc.tensor.matmul(pt1[:], w3t[kb][:, mt * 128:(mt + 1) * 128],
                                         Gsb[kb][:], start=(kb == 0), stop=(kb == 1))
                    t1w = fpool.tile([128, C], F32, name="t1w", tag="t1w")
                    nc.vector.tensor_tensor(t1w[:], pt1[:], w3n[mt][:], AX.mult)
                    nc.vector.reduce_sum(st3[:, 8 + mt:9 + mt], t1w[:], X_AXIS)

            tot3 = wpool.tile([128, 16], F32, name="tot3")
            stats_collective(st3, 16, tot3, "bn3")
            s3c = wpool.tile([128, 8], F32, name="s3c")
            c3c = wpool.tile([128, 8], F32, name="c3c")
            bn_coeffs(tot3, g3t, b3t, CNT2, 8, s3c, c3c, "bn3")

            # W3 matmuls staged to SBUF (no coeff dependency, so the PE can
            # fill the BN3-collective window), then the finale drains:
            # out = relu(s*y3 + c + x)
            with tc.tile_pool(name="y3pool", bufs=1) as y3pool, \
                 tc.tile_pool(name="psum6", bufs=4, space="PSUM") as psum6:
                Y3 = [y3pool.tile([128, NQ], F32, name=f"y3_{mt}")
                      for mt in range(8)]
                for mt in range(8):
                    for ci, (off, sz) in enumerate(CH3):
                        ps = psum6.tile([128, sz], F32, name="pw3", tag="pw3")
                        for k in range(2):
                            nc.tensor.matmul(ps[:], w3t[k][:, mt * 128:(mt + 1) * 128],
                                             OUT2[k][:, off:off + sz],
                                             start=(k == 0), stop=(k == 1))
                        nc.scalar.activation(Y3[mt][:, off:off + sz], ps[:], AF.Copy)
                # drain in full rows: fewer, larger ops amortize the
                # per-instruction sync overhead
                for mt in range(8):
                    tf = fpool.tile([128, NQ], F32, name="tf", tag="tf")
                    nc.vector.scalar_tensor_tensor(tf[:], Y3[mt][:],
                                                   s3c[:, mt:mt + 1],
                                                   XR[mt][:], AX.mult, AX.add)
                    to = fpool.tile([128, NQ], F32, name="to", tag="to")
                    nc.scalar.activation(to[:], tf[:], AF.Relu,
                                         bias=c3c[:, mt:mt + 1])
                    eng = (nc.sync, nc.gpsimd, nc.scalar)[mt % 3]
                    eng.dma_start(OUTd[mt * 128:(mt + 1) * 128, :], to[:])


_NC_CACHE = {}


def _get_program():
    if "nc" not in _NC_CACHE:
        _NC_CACHE["nc"] = build_program()
    return _NC_CACHE["nc"]


def _host_prep(inputs):
    x = np.ascontiguousarray(inputs["x"].reshape(B, CIN, N))
    rel = (inputs["rel_h"] + inputs["rel_w"] + inputs["rel_d"]).reshape(HEADS, DH, N)
    rel = np.ascontiguousarray(rel.astype(np.float32))
    W1T = np.ascontiguousarray(inputs["W1"].T.astype(np.float32))
    WQT = np.ascontiguousarray(inputs["Wq"].T.astype(np.float32))
    WKT = np.ascontiguousarray(inputs["Wk"].T.astype(np.float32))
    WVT = np.ascontiguousarray(inputs["Wv"].T.astype(np.float32))
    W3T = np.ascontiguousarray(inputs["W3"].T.astype(np.float32))
    W3N = np.ascontiguousarray(inputs["W3"].astype(np.float32))
    WKQ = np.stack([np.concatenate([WKT[:, h * DH:(h + 1) * DH],
                                    WQT[:, h * DH:(h + 1) * DH]], axis=1)
                    for h in range(HEADS)]).astype(np.float32)
    BQ = inputs["bq"].reshape(HEADS, DH).astype(np.float32)
    IDT = np.eye(128, dtype=np.float32)
    GB1 = np.stack([inputs["g1"], inputs["b1"]]).astype(np.float32)
    GB2 = np.stack([inputs["g2"], inputs["b2"]]).astype(np.float32)
    GB3 = np.stack([inputs["g3"], inputs["b3"]]).astype(np.float32)

    in_maps = []
    for c in range(N_CORES):
        b, s = c // 2, c % 2
        xb = np.roll(x[b], -s * NQ, axis=1)
        relc = np.ascontiguousarray(rel[:, :, s * NQ:(s + 1) * NQ])
        in_maps.append({
            "X": np.ascontiguousarray(xb), "W1T": W1T, "WQT": WQT,
            "WVT": WVT, "W3T": W3T, "W3N": W3N, "WKQ": WKQ, "REL": relc,
            "BQ": BQ, "IDT": IDT, "GB1": GB1, "GB2": GB2, "GB3": GB3,
        })
    return in_maps


def run(inputs, trace=False, trace_kwargs=None):
    from concourse import bass_utils
    nc = _get_program()
    in_maps = _host_prep(inputs)
    res = bass_utils.run_bass_kernel_spmd(
        nc, in_maps, core_ids=list(range(N_CORES)), trace=trace,
        **(trace_kwargs or {}))
    out = np.empty((B, CIN, N), np.float32)
    for c in range(N_CORES):
        b, s = c // 2, c % 2
        out[b, :, s * NQ:(s + 1) * NQ] = res.results[c]["OUT"]
    return out.reshape(B, CIN, 14, 14, 14), res


def kernel(**inputs):
    out, _ = run(inputs, trace=False)
    return out


# revision 37
# speedup vs baseline: 1.0701x; 1.0701x over previous
"""Bottleneck-MHSA fused kernel for 8 Trainium2 NeuronCores.

Sharding: core c = 2*b + s handles batch b; attention queries are split in
half between the two cores of a pair. Each core computes conv1 + BN1 + qkv
for its whole batch (redundantly with its pair partner), then attention for
all 4 heads over its query half, then BN2 + W3 + BN3 + residual for its
query half. Cross-core traffic is only the three tiny BN statistics
AllReduces.

Key optimizations over the straightforward version:
- bv and bk are dropped: BN2 absorbs per-channel shifts (bv), and bk only
  adds a per-query constant to scores, which softmax cancels.
- conv1 stats (sum/sumsq) are fused into the PSUM->SBUF drain on the
  Scalar engine via activation accum_out; DVE does no work in conv1.
- BN applies are single Scalar activation ops Relu(scale*x+bias).
- rsqrt via DVE Newton iteration, so the Scalar act table never swaps
  (exp/copy/relu/square/identity live in one table).
- softmax denominator reciprocal runs on 64 partitions (broadcast first).
- BN3 stats come from the Gram matrix W3 (out2 out2^T) W3^T diagonal,
  computed *before* the W3 matmul so the last collective overlaps it and
  the finale drains PSUM directly (no y3 staging).
"""
import numpy as np

HEADS = 4
DH = 64
C = 256          # PLANES
CIN = 1024       # IN_PLANES
N = 2744         # tokens per batch
NQ = N // 2      # query half per core
B = 4
EPS = 1e-5
N_CORES = 8
CNT12 = 8 * N    # BN1 effective count (pairs double-count; mean/var exact)
CNT2 = 8 * NQ    # BN2/BN3 count (distinct shards)


def _chunks(total, n):
    # even sizes (fp32r matmul requires an even moving free dim)
    assert total % 2 == 0
    half = total // 2
    sizes = [2 * (half // n + (1 if i < half % n else 0)) for i in range(n)]
    out, off = [], 0
    for s in sizes:
        out.append((off, s))
        off += s
    return out


CH6 = _chunks(N, 6)     # key/token chunks (456/458 wide, all >=256 for f32r)
CH3 = _chunks(NQ, 3)    # query chunks
MT22 = [(t * 128, min(128, N - t * 128)) for t in range((N + 127) // 128)]
TQ11 = [(t * 128, min(128, NQ - t * 128)) for t in range((NQ + 127) // 128)]


def build_program():
    from concourse import bacc, mybir, tile

    F32 = mybir.dt.float32
    F32R = mybir.dt.float32r
    I32 = mybir.dt.int32
    BF16 = mybir.dt.bfloat16

    nc = bacc.Bacc("TRN2", target_bir_lowering=False, debug=False,
                   num_devices=N_CORES)

    # ---- I/O ----
    io = {}
    io["X"] = nc.dram_tensor("X", [CIN, N], F32R, kind="ExternalInput").ap()
    io["W1T"] = nc.dram_tensor("W1T", [CIN, C], F32R, kind="ExternalInput").ap()
    io["WQT"] = nc.dram_tensor("WQT", [C, C], F32R, kind="ExternalInput").ap()
    io["WVT"] = nc.dram_tensor("WVT", [C, C], F32R, kind="ExternalInput").ap()
    io["W3T"] = nc.dram_tensor("W3T", [C, CIN], F32R, kind="ExternalInput").ap()
    io["W3N"] = nc.dram_tensor("W3N", [CIN, C], F32, kind="ExternalInput").ap()
    io["WKQ"] = nc.dram_tensor("WKQ", [HEADS, C, 128], F32R, kind="ExternalInput").ap()
    io["REL"] = nc.dram_tensor("REL", [HEADS, DH, NQ], F32R, kind="ExternalInput").ap()
    io["BQ"] = nc.dram_tensor("BQ", [HEADS, DH], F32, kind="ExternalInput").ap()
    io["IDT"] = nc.dram_tensor("IDT", [128, 128], F32R, kind="ExternalInput").ap()
    io["GB1"] = nc.dram_tensor("GB1", [2, C], F32, kind="ExternalInput").ap()
    io["GB2"] = nc.dram_tensor("GB2", [2, C], F32, kind="ExternalInput").ap()
    io["GB3"] = nc.dram_tensor("GB3", [2, CIN], F32, kind="ExternalInput").ap()
    io["OUT"] = nc.dram_tensor("OUT", [CIN, NQ], F32, kind="ExternalOutput").ap()

    with tile.TileContext(nc) as tc:
        _emit(nc, tc, mybir, F32, F32R, I32, BF16, io)

    nc.compile()
    from concourse.bass_interp import get_hw_module
    nc.m = get_hw_module(nc.m)
    return nc


def _emit(nc, tc, mybir, F32, F32R, I32, BF16, io):
    import contextlib

    AX = mybir.AluOpType
    AF = mybir.ActivationFunctionType
    X_AXIS = mybir.AxisListType.X

    Xd, W1T, WQT, WVT, W3T = io["X"], io["W1T"], io["WQT"], io["WVT"], io["W3T"]
    W3N, WKQ, RELd, BQd, IDTd = io["W3N"], io["WKQ"], io["REL"], io["BQ"], io["IDT"]
    GB1, GB2, GB3, OUTd = io["GB1"], io["GB2"], io["GB3"], io["OUT"]

    def stats_collective(src_sbuf, width, out_gst, tag):
        """AllReduce [P, width] partials over all 8 cores."""
        p = src_sbuf.shape[0]
        cin = dpool.tile([p, width], F32, name=f"ccin_{tag}")
        cout = dpool.tile([p, width], F32, addr_space="Shared",
                          name=f"ccout_{tag}")
        nc.sync.dma_start(cin[:], src_sbuf[:])
        nc.gpsimd.collective_compute(
            "AllReduce", AX.add,
            replica_groups=[list(range(N_CORES))],
            ins=[cin.opt()], outs=[cout.opt()],
        )
        nc.sync.dma_start(out_gst[:], cout[:])

    def rsqrt_newton(y, x, tag):
        """y = 1/sqrt(x) on DVE only (bit-trick seed + 2 Newton steps)."""
        p, w = x.shape[0], x.shape[1]
        xi = x[:].bitcast(I32)
        t1 = wpool.tile([p, w], I32, name=f"rsq_t1_{tag}")
        nc.vector.tensor_scalar(t1[:], xi, 1, None, AX.arith_shift_right)
        yi = y[:].bitcast(I32)
        nc.vector.tensor_scalar(yi, t1[:], -1, 0x5f3759df, AX.mult, AX.add)
        h = wpool.tile([p, w], F32, name=f"rsq_h_{tag}")
        for _ in range(2):
            nc.vector.tensor_tensor(h[:], y[:], y[:], AX.mult)
            nc.vector.tensor_tensor(h[:], x[:], h[:], AX.mult)
            nc.vector.tensor_scalar(h[:], h[:], -0.5, 1.5, AX.mult, AX.add)
            nc.vector.tensor_tensor(y[:], y[:], h[:], AX.mult)

    def bn_coeffs(tot, gt, bt, cnt, w, sc, cc, tag):
        """tot [P, 2w] = [sums | sumsqs] -> scale sc [P, w], bias cc [P, w]."""
        p = tot.shape[0]
        mean = wpool.tile([p, w], F32, name=f"mean_{tag}")
        var = wpool.tile([p, w], F32, name=f"var_{tag}")
        nc.vector.tensor_scalar_mul(mean[:], tot[:, 0:w], 1.0 / cnt)
        nc.vector.tensor_scalar_mul(var[:], tot[:, w:2 * w], 1.0 / cnt)
        m2 = wpool.tile([p, w], F32, name=f"m2_{tag}")
        nc.vector.tensor_tensor(m2[:], mean[:], mean[:], AX.mult)
        nc.vector.tensor_tensor(var[:], var[:], m2[:], AX.subtract)
        nc.vector.tensor_scalar_add(var[:], var[:], EPS)
        rstd = wpool.tile([p, w], F32, name=f"rstd_{tag}")
        rsqrt_newton(rstd, var, tag)
        nc.vector.tensor_tensor(sc[:], gt[:], rstd[:], AX.mult)
        tmp = wpool.tile([p, w], F32, name=f"tmpc_{tag}")
        nc.vector.tensor_tensor(tmp[:], sc[:], mean[:], AX.mult)
        nc.vector.tensor_tensor(cc[:], bt[:], tmp[:], AX.subtract)

    with contextlib.ExitStack() as top:
        wpool = top.enter_context(tc.tile_pool(name="wpool", bufs=1))
        dpool = top.enter_context(tc.tile_pool(name="dpool", bufs=1, space="DRAM"))

        # f32r constants (memset to f32r is rejected at codegen; go via f32)
        onespad = wpool.tile([128, 128], F32, name="onespad")
        nc.vector.memset(onespad[:], 1.0)

        # ---- startup barrier, as early as possible: the first collective
        # pays ~66us of CC firmware warmup per core, so trigger it
        # immediately and let it warm up under conv1 ----
        bar_in = dpool.tile([1, 2], F32, name="bar_in")
        bar_out = dpool.tile([1, 2], F32, addr_space="Shared", name="bar_out")
        nc.gpsimd.dma_start(bar_in[:], onespad[0:1, 0:2])
        nc.gpsimd.collective_compute(
            "AllReduce", AX.add,
            replica_groups=[list(range(N_CORES))],
            ins=[bar_in.opt()], outs=[bar_out.opt()],
        )

        onesr = wpool.tile([128, DH], F32R, name="onesr")
        nc.vector.tensor_copy(onesr[:], onespad[:, 0:DH])

        # ---- phase-1 weights only; the rest loads after the conv loop so X
        # gets the DMA bandwidth ----
        w1t = []
        for k in range(8):
            t = wpool.tile([128, C], F32R, name=f"w1t{k}")
            nc.scalar.dma_start(t[:], W1T[k * 128:(k + 1) * 128, :])
            w1t.append(t)

        def warm_pe(n, tag):
            # keep the PE busy through idle windows: the clock ramps to max
            # only after ~3us of sustained work, and idle gaps reset it
            with tc.tile_pool(name=f"warm_{tag}", bufs=1, space="PSUM") as wp:
                pw = wp.tile([128, 256], F32, name=f"pw_{tag}")
                for _ in range(n):
                    nc.tensor.matmul(pw[:], w1t[0][:, 0:128], w1t[0][:, 0:256],
                                     start=True, stop=True)
                sink = wpool.tile([1, 2], F32, name=f"wsink_{tag}")
                nc.scalar.activation(sink[:], pw[0:1, 0:2], AF.Copy)

        warm_pe(20, "a")
        g1t = wpool.tile([128, 2], F32, name="g1t")
        b1t = wpool.tile([128, 2], F32, name="b1t")
        nc.scalar.dma_start(g1t[:], GB1[0].rearrange("(m p) -> p m", p=128))
        nc.scalar.dma_start(b1t[:], GB1[1].rearrange("(m p) -> p m", p=128))

        # stats accumulators
        S1 = wpool.tile([128, 12], F32, name="S1")   # conv1 sums   (mt*6+ci)
        Q1 = wpool.tile([128, 12], F32, name="Q1")   # conv1 sumsqs
        S2 = wpool.tile([DH, 16], F32, name="S2")    # attn sums    (h*4+ci)
        nc.vector.memset(S2[:], 0.0)
        Q2 = wpool.tile([DH, 16], F32, name="Q2")
        nc.vector.memset(Q2[:], 0.0)

        OUT2 = [wpool.tile([128, NQ], F32R, name=f"out2_{m}") for m in range(2)]

        with contextlib.ExitStack() as ph_a:
            qpool = ph_a.enter_context(tc.tile_pool(name="qpool", bufs=1))
            KHAT = [qpool.tile([128, N], F32R, name=f"khat{h}") for h in range(HEADS)]
            QHAT = [qpool.tile([128, NQ], F32R, name=f"qhat{h}") for h in range(HEADS)]
            # one VTON tile for all heads: head h block = cols [h*1430, h*1430+1430)
            # laid out as 22 tiles of [v(64) | ones(1)]; bf16 is enough for
            # softmax-side values and halves the copy cost
            VTON = qpool.tile([128, HEADS * 22 * 65], BF16, name="vton")
            vt4 = VTON[:].rearrange("p (h t c) -> p h t c", h=HEADS, c=65)
            # ones columns for all heads/tiles in one strided write
            nc.vector.tensor_copy(
                vt4[:, :, :, 64],
                onespad[:, 0:HEADS * 22].rearrange("p (h t) -> p h t", h=HEADS))
            for h in range(HEADS):
                nc.scalar.dma_start(QHAT[h][DH:128, :], RELd[h])

            with contextlib.ExitStack() as ph1:
                y1pool = ph1.enter_context(tc.tile_pool(name="y1pool", bufs=1))
                Y1 = [y1pool.tile([128, N], F32R, name=f"y1_{m}") for m in range(2)]

                # ---- phase 1: conv1 (y1 = W1 @ x), stats fused into the
                # Scalar-engine PSUM drain (copy+accum, square+accum).
                # X loads in 914-wide tiles: >=3.6KB per partition line keeps
                # the DMA engines at full rate ----
                XCH = [(0, 914), (914, 914), (1828, 916)]
                with tc.tile_pool(name="xpool", bufs=2) as xpool, \
                     tc.tile_pool(name="psum1", bufs=3, space="PSUM") as psum1:
                    for c3, (xoff, xsz) in enumerate(XCH):
                        xts = []
                        for k in range(8):
                            t = xpool.tile([128, xsz], F32R, name=f"xc{k}",
                                           tag=f"xc{k}")
                            eng = nc.sync if k % 2 == 0 else nc.gpsimd
                            eng.dma_start(t[:], Xd[k * 128:(k + 1) * 128,
                                                   xoff:xoff + xsz])
                            xts.append(t)
                        # sub-chunk offsets must be even (f32r alignment)
                        halves = [(0, xsz // 2 - (xsz // 2) % 2)]
                        halves.append((halves[0][1], xsz - halves[0][1]))
                        for half, (hoff, hsz) in enumerate(halves):
                            for mt in range(2):
                                ps = psum1.tile([128, hsz], F32, name="pconv",
                                                tag="pconv", padded_shape=[128, 458])
                                for k in range(8):
                                    nc.tensor.matmul(
                                        ps[:], w1t[k][:, mt * 128:(mt + 1) * 128],
                                        xts[k][:, hoff:hoff + hsz],
                                        start=(k == 0), stop=(k == 7))
                                off = xoff + hoff
                                idx = mt * 6 + (c3 * 2 + half)
                                nc.scalar.activation(Y1[mt][:, off:off + hsz], ps[:],
                                                     AF.Copy,
                                                     accum_out=S1[:, idx:idx + 1])
                                sq = xpool.tile([128, hsz], F32, name="sqs",
                                                tag="sqs", padded_shape=[128, 458])
                                nc.scalar.activation(sq[:], ps[:], AF.Square,
                                                     accum_out=Q1[:, idx:idx + 1])

                # ---- remaining weights (deferred so X had DMA priority) ----
                wqt, wvt = [], []
                for srcw, dst, nm in ((WQT, wqt, "wq"), (WVT, wvt, "wv")):
                    for k in range(2):
                        t = wpool.tile([128, C], F32R, name=f"{nm}{k}")
                        nc.scalar.dma_start(t[:], srcw[k * 128:(k + 1) * 128, :])
                        dst.append(t)
                wkqt = []
                for h in range(HEADS):
                    row = []
                    for k in range(2):
                        t = wpool.tile([128, 128], F32R, name=f"wkq{h}_{k}")
                        nc.gpsimd.dma_start(t[:], WKQ[h][k * 128:(k + 1) * 128, :])
                        row.append(t)
                    wkqt.append(row)
                bqt = wpool.tile([DH, HEADS], F32, name="bqt")
                nc.scalar.dma_start(bqt[:], BQd[:].rearrange("h d -> d h"))
                g2t = wpool.tile([DH, HEADS], F32, name="g2t")
                b2t = wpool.tile([DH, HEADS], F32, name="b2t")
                nc.scalar.dma_start(g2t[:], GB2[0].rearrange("(h d) -> d h", d=DH))
                nc.scalar.dma_start(b2t[:], GB2[1].rearrange("(h d) -> d h", d=DH))
                w3t = []
                for k in range(2):
                    t = wpool.tile([128, CIN], F32R, name=f"w3t{k}")
                    nc.sync.dma_start(t[:], W3T[k * 128:(k + 1) * 128, :])
                    w3t.append(t)
                w3n = []
                for mt in range(8):
                    t = wpool.tile([128, C], F32, name=f"w3n{mt}")
                    nc.sync.dma_start(t[:], W3N[mt * 128:(mt + 1) * 128, :])
                    w3n.append(t)
                idt = wpool.tile([128, 128], F32R, name="idt")
                nc.sync.dma_start(idt[:], IDTd[:])
                g3t = wpool.tile([128, 8], F32, name="g3t")
                b3t = wpool.tile([128, 8], F32, name="b3t")
                nc.sync.dma_start(g3t[:], GB3[0].rearrange("(m p) -> p m", p=128))
                nc.sync.dma_start(b3t[:], GB3[1].rearrange("(m p) -> p m", p=128))

                # ---- phase 1b: BN1 stats collective + coeffs ----
                s1sum = wpool.tile([128, 2], F32, name="s1sum")
                q1sum = wpool.tile([128, 2], F32, name="q1sum")
                nc.vector.reduce_sum(s1sum[:], S1[:].rearrange("p (m c) -> p m c", c=6), X_AXIS)
                nc.vector.reduce_sum(q1sum[:], Q1[:].rearrange("p (m c) -> p m c", c=6), X_AXIS)
                st1 = wpool.tile([128, 4], F32, name="st1")
                nc.vector.tensor_copy(st1[:, 0:2], s1sum[:])
                nc.vector.tensor_copy(st1[:, 2:4], q1sum[:])
                tot1 = wpool.tile([128, 4], F32, name="tot1")
                stats_collective(st1, 4, tot1, "bn1")
                s1c = wpool.tile([128, 2], F32, name="s1c")
                c1c = wpool.tile([128, 2], F32, name="c1c")
                bn_coeffs(tot1, g1t, b1t, CNT12, 2, s1c, c1c, "bn1")

                warm_pe(24, "b")

                # ---- phase 2: out1 = relu(s*y1 + c), in place, Scalar ----
                OUT1 = [Y1[m][:] for m in range(2)]
                for (off, sz) in CH6:
                    for mt in range(2):
                        nc.scalar.activation(OUT1[mt][:, off:off + sz],
                                             Y1[mt][:, off:off + sz],
                                             AF.Relu,
                                             bias=c1c[:, mt:mt + 1],
                                             scale=s1c[:, mt:mt + 1])

                # ---- phase 3a: vT = out1^T @ WvT (no bias; BN2 absorbs bv) ----
                with tc.tile_pool(name="psum3a", bufs=3, space="PSUM") as psum3a:
                    for t, (mo, msz) in enumerate(MT22):
                        ps = psum3a.tile([128, C], F32, name="pvt", tag="pvt")
                        for k in range(2):
                            nc.tensor.matmul(ps[0:msz, :], OUT1[k][:, mo:mo + msz],
                                             wvt[k][:], start=(k == 0), stop=(k == 1))
                        nc.vector.tensor_copy(
                            vt4[0:msz][:, :, t, 0:64],
                            ps[0:msz, :].rearrange("p (h d) -> p h d", h=HEADS))

                # ---- phase 3b: KHAT = [k; q] raw (biases cancel in softmax),
                # QHAT q-half with bq ----
                with tc.tile_pool(name="psum3b", bufs=3, space="PSUM") as psum3b:
                    for h in range(HEADS):
                        hs = h * DH
                        for (off, sz) in CH6:
                            ps = psum3b.tile([128, sz], F32, name="pkh", tag="pkh")
                            for k in range(2):
                                nc.tensor.matmul(ps[:], wkqt[h][k][:],
                                                 OUT1[k][:, off:off + sz],
                                                 start=(k == 0), stop=(k == 1))
                            nc.scalar.activation(KHAT[h][:, off:off + sz], ps[:],
                                                 AF.Copy)
                        for (off, sz) in CH3:
                            pq = psum3b.tile([DH, sz], F32, name="pqh", tag="pqh")
                            for k in range(2):
                                nc.tensor.matmul(pq[:], wqt[k][:, hs:hs + DH],
                                                 OUT1[k][:, off:off + sz],
                                                 start=(k == 0), stop=(k == 1))
                            nc.scalar.activation(QHAT[h][0:DH, off:off + sz], pq[:],
                                                 AF.Identity, bias=bqt[:, h:h + 1])

            # ---- phase 4: attention (S^T layout, staged exp, fused denom) ----
            with tc.tile_pool(name="oattp", bufs=1) as oattp, \
                 tc.tile_pool(name="epool", bufs=2) as epool, \
                 tc.tile_pool(name="psum4", bufs=1, space="PSUM") as psum4:
                OATT = [oattp.tile([DH, NQ], F32R, name=f"oatt{h}") for h in range(HEADS)]
                QP = [(0, 1024, [(0, 512), (512, 512)]),
                      (1024, 348, [(0, 348)])]
                for h in range(HEADS):
                    for qo, qw, subs in QP:
                        # pav0 double-buffered so the next group's AV matmuls
                        # don't wait on this group's softmax-denominator drain
                        pavs = [psum4.tile([65, sz], F32, name=f"pav{si}",
                                           tag=f"pav{si}", bufs=(2 if si == 0 else 1))
                                for si, (so, sz) in enumerate(subs)]
                        for t, (mo, msz) in enumerate(MT22):
                            ps = psum4.tile([128, qw], F32, name="ps", tag="ps", bufs=2,
                                            padded_shape=[128, 1024])
                            for so, sz in subs:
                                nc.tensor.matmul(ps[0:msz, so:so + sz],
                                                 KHAT[h][:, mo:mo + msz],
                                                 QHAT[h][:, qo + so:qo + so + sz],
                                                 start=True, stop=True)
                            e = epool.tile([128, qw], BF16, name="e", tag="e", bufs=6)
                            nc.scalar.activation(e[0:msz, :], ps[0:msz, :], AF.Exp)
                            for si, (so, sz) in enumerate(subs):
                                nc.tensor.matmul(pavs[si][:],
                                                 VTON[0:msz, h * 1430 + 65 * t:
                                                      h * 1430 + 65 * t + 65],
                                                 e[0:msz, so:so + sz],
                                                 start=(t == 0), stop=(t == 21))
                        for si, (so, sz) in enumerate(subs):
                            pav = pavs[si]
                            off = qo + so
                            # denominator row -> SBUF (Scalar), broadcast to 64
                            # partitions (PE), then reciprocal on 64 lanes (DVE)
                            # on DVE: the Scalar queue is backed up with EXPs,
                            # which would stall the pb matmul ~3.5us
                            den = epool.tile([65, sz], F32R, name="den", tag="den",
                                             bufs=2)
                            nc.vector.tensor_copy(den[DH:65, :], pav[DH:65, :])
                            pb = psum4.tile([DH, sz], F32, name="pb", tag="pb", bufs=1)
                            nc.tensor.matmul(pb[:], onesr[DH:65, :], den[DH:65, :],
                                             start=True, stop=True)
                            pbs = epool.tile([DH, sz], F32R, name="pbs", tag="pbs", bufs=2)
                            with nc.allow_low_precision(reason="softmax denom recip"):
                                nc.vector.reciprocal(pbs[:], pb[:])
                            nc.vector.tensor_tensor(OATT[h][:, off:off + sz],
                                                    pav[0:DH, :], pbs[:], AX.mult)
                            idx = h * 4 + (0 if qo == 0 else 2) + si
                            nc.vector.reduce_sum(S2[:, idx:idx + 1],
                                                 OATT[h][:, off:off + sz], X_AXIS)
                            sq2 = epool.tile([DH, sz], F32, name="sq2", tag="sq2", bufs=2)
                            nc.vector.tensor_tensor(sq2[:], OATT[h][:, off:off + sz],
                                                    OATT[h][:, off:off + sz], AX.mult)
                            nc.vector.reduce_sum(Q2[:, idx:idx + 1], sq2[:], X_AXIS)

                # ---- phase 5: BN2 + relu (Scalar), assemble OUT2 ----
                s2sum = wpool.tile([DH, 4], F32, name="s2sum")
                q2sum = wpool.tile([DH, 4], F32, name="q2sum")
                nc.vector.reduce_sum(s2sum[:], S2[:].rearrange("p (h c) -> p h c", c=4), X_AXIS)
                nc.vector.reduce_sum(q2sum[:], Q2[:].rearrange("p (h c) -> p h c", c=4), X_AXIS)
                st2 = wpool.tile([DH, 8], F32, name="st2")
                nc.vector.tensor_copy(st2[:, 0:4], s2sum[:])
                nc.vector.tensor_copy(st2[:, 4:8], q2sum[:])
                tot2 = wpool.tile([DH, 8], F32, name="tot2")
                stats_collective(st2, 8, tot2, "bn2")
                s2c = wpool.tile([DH, 4], F32, name="s2c")
                c2c = wpool.tile([DH, 4], F32, name="c2c")
                bn_coeffs(tot2, g2t, b2t, CNT2, 4, s2c, c2c, "bn2")
                for h in range(HEADS):
                    nc.scalar.activation(OATT[h][:], OATT[h][:], AF.Relu,
                                         bias=c2c[:, h:h + 1],
                                         scale=s2c[:, h:h + 1])
                    nc.scalar.dma_start(
                        OUT2[h // 2][(h % 2) * DH:(h % 2) * DH + DH, :], OATT[h][:])

        # ---- phase 6: BN3 stats via Gram (before W3!), then W3 + finale ----
        with tc.tile_pool(name="otpool", bufs=1) as otpool, \
             tc.tile_pool(name="fpool", bufs=2) as fpool:
            XR = []
            for mt in range(8):
                xr = fpool.tile([128, NQ], F32, name=f"xr{mt}", tag=f"xr{mt}", bufs=1)
                eng = (nc.sync, nc.gpsimd, nc.scalar)[mt % 3]
                eng.dma_start(xr[:], Xd[mt * 128:(mt + 1) * 128, 0:NQ].bitcast(F32))
                XR.append(xr)

            warm_pe(20, "c")

            # out2^T chunks [tok, 256] via PE transpose
            OT = [otpool.tile([128, C], F32R, name=f"ot{t}") for t in range(11)]
            with tc.tile_pool(name="psumT", bufs=4, space="PSUM") as psumT:
                for t, (qo, qsz) in enumerate(TQ11):
                    for mt in range(2):
                        pt = psumT.tile([qsz, 128], F32R, name="pt", tag="pt",
                                        bufs=2, padded_shape=[128, 128])
                        nc.tensor.transpose(pt[:], OUT2[mt][:, qo:qo + qsz],
                                            idt[:])
                        nc.scalar.activation(OT[t][0:qsz, mt * 128:(mt + 1) * 128],
                                             pt[:], AF.Copy)
                # token sums of out2 for the y3 channel sums; zero-padded
                # columns keep the matvec free dim even (f32r requirement)
                sv = wpool.tile([128, 4], F32R, name="sv")
                nc.vector.memset(sv[:].bitcast(F32), 0.0)
                with nc.allow_low_precision(reason="f32r view of f32 sums"):
                    for mt in range(2):
                        nc.vector.reduce_sum(sv[:, 2 * mt:2 * mt + 1],
                                             OUT2[mt][:], X_AXIS)
                # Gram G = out2 @ out2^T, [256,256] as 2 x [128,256]
                Gsb = [wpool.tile([128, C], F32R, name=f"gsb{kb}") for kb in range(2)]
                for kb in range(2):
                    pg = psumT.tile([128, C], F32, name="pg", tag="pg", bufs=1)
                    for t, (qo, qsz) in enumerate(TQ11):
                        nc.tensor.matmul(pg[:], OT[t][0:qsz, kb * 128:(kb + 1) * 128],
                                         OT[t][0:qsz, :],
                                         start=(t == 0), stop=(t == 10))
                    nc.scalar.activation(Gsb[kb][:], pg[:], AF.Copy)

            st3 = wpool.tile([128, 16], F32, name="st3")
            with tc.tile_pool(name="psumG", bufs=2, space="PSUM") as psumG:
                # sums: y3_sum[o] = W3 @ sum_n(out2), computed directly in
                # [128, 8] channel layout (out partition = channel-in-block)
                pstats = psumG.tile([128, 16], F32, name="pstats", tag="pstats",
                                    bufs=1)
                for mt in range(8):
                    for kb in range(2):
                        nc.tensor.matmul(pstats[:, 2 * mt:2 * mt + 2],
                                         w3t[kb][:, mt * 128:(mt + 1) * 128],
                                         sv[:, 2 * kb:2 * kb + 2],
                                         start=(kb == 0), stop=(kb == 1))
                nc.scalar.activation(
                    st3[:, 0:8],
                    pstats[:].rearrange("p (c two) -> p c two", two=2)[:, :, 0],
                    AF.Copy)
                # sumsqs: diag(W3 G W3^T) = rowsum((W3 @ G) * W3)
                for mt in range(8):
                    pt1 = psumG.tile([128, C], F32, name="pt1", tag="pt1", bufs=2)
                    for kb in range(2):
                        nc.tensor.matmul(pt1[:], w3t[kb][:, mt * 128:(mt + 1) * 128],
                                         Gsb[kb][:], start=(kb == 0), stop=(kb == 1))
                    t1w = fpool.tile([128, C], F32, name="t1w", tag="t1w")
                    nc.vector.tensor_tensor(t1w[:], pt1[:], w3n[mt][:], AX.mult)
                    nc.vector.reduce_sum(st3[:, 8 + mt:9 + mt], t1w[:], X_AXIS)

            tot3 = wpool.tile([128, 16], F32, name="tot3")
            stats_collective(st3, 16, tot3, "bn3")
            s3c = wpool.tile([128, 8], F32, name="s3c")
            c3c = wpool.tile([128, 8], F32, name="c3c")
            bn_coeffs(tot3, g3t, b3t, CNT2, 8, s3c, c3c, "bn3")

            # W3 matmuls staged to SBUF (no coeff dependency, so the PE can
            # fill the BN3-collective window), then the finale drains:
            # out = relu(s*y3 + c + x)
            with tc.tile_pool(name="y3pool", bufs=1) as y3pool, \
                 tc.tile_pool(name="psum6", bufs=4, space="PSUM") as psum6:
                Y3 = [y3pool.tile([128, NQ], F32, name=f"y3_{mt}")
                      for mt in range(8)]
                for mt in range(8):
                    for ci, (off, sz) in enumerate(CH3):
                        ps = psum6.tile([128, sz], F32, name="pw3", tag="pw3")
                        for k in range(2):
                            nc.tensor.matmul(ps[:], w3t[k][:, mt * 128:(mt + 1) * 128],
                                             OUT2[k][:, off:off + sz],
                                             start=(k == 0), stop=(k == 1))
                        nc.scalar.activation(Y3[mt][:, off:off + sz], ps[:], AF.Copy)
                # drain in full rows: fewer, larger ops amortize the
                # per-instruction sync overhead
                for mt in range(8):
                    tf = fpool.tile([128, NQ], F32, name="tf", tag="tf")
                    nc.vector.scalar_tensor_tensor(tf[:], Y3[mt][:],
                                                   s3c[:, mt:mt + 1],
                                                   XR[mt][:], AX.mult, AX.add)
                    to = fpool.tile([128, NQ], F32, name="to", tag="to")
                    nc.scalar.activation(to[:], tf[:], AF.Relu,
                                         bias=c3c[:, mt:mt + 1])
                    eng = (nc.sync, nc.gpsimd, nc.scalar)[mt % 3]
                    eng.dma_start(OUTd[mt * 128:(mt + 1) * 128, :], to[:])


_NC_CACHE = {}


def _get_program():
    if "nc" not in _NC_CACHE:
        _NC_CACHE["nc"] = build_program()
    return _NC_CACHE["nc"]


def _host_prep(inputs):
    x = np.ascontiguousarray(inputs["x"].reshape(B, CIN, N))
    rel = (inputs["rel_h"] + inputs["rel_w"] + inputs["rel_d"]).reshape(HEADS, DH, N)
    rel = np.ascontiguousarray(rel.astype(np.float32))
    W1T = np.ascontiguousarray(inputs["W1"].T.astype(np.float32))
    WQT = np.ascontiguousarray(inputs["Wq"].T.astype(np.float32))
    WKT = np.ascontiguousarray(inputs["Wk"].T.astype(np.float32))
    WVT = np.ascontiguousarray(inputs["Wv"].T.astype(np.float32))
    W3T = np.ascontiguousarray(inputs["W3"].T.astype(np.float32))
    W3N = np.ascontiguousarray(inputs["W3"].astype(np.float32))
    WKQ = np.stack([np.concatenate([WKT[:, h * DH:(h + 1) * DH],
                                    WQT[:, h * DH:(h + 1) * DH]], axis=1)
                    for h in range(HEADS)]).astype(np.float32)
    BQ = inputs["bq"].reshape(HEADS, DH).astype(np.float32)
    IDT = np.eye(128, dtype=np.float32)
    GB1 = np.stack([inputs["g1"], inputs["b1"]]).astype(np.float32)
    GB2 = np.stack([inputs["g2"], inputs["b2"]]).astype(np.float32)
    GB3 = np.stack([inputs["g3"], inputs["b3"]]).astype(np.float32)

    in_maps = []
    for c in range(N_CORES):
        b, s = c // 2, c % 2
        xb = np.roll(x[b], -s * NQ, axis=1)
        relc = np.ascontiguousarray(rel[:, :, s * NQ:(s + 1) * NQ])
        in_maps.append({
            "X": np.ascontiguousarray(xb), "W1T": W1T, "WQT": WQT,
            "WVT": WVT, "W3T": W3T, "W3N": W3N, "WKQ": WKQ, "REL": relc,
            "BQ": BQ, "IDT": IDT, "GB1": GB1, "GB2": GB2, "GB3": GB3,
        })
    return in_maps


def run(inputs, trace=False, trace_kwargs=None):
    from concourse import bass_utils
    nc = _get_program()
    in_maps = _host_prep(inputs)
    res = bass_utils.run_bass_kernel_spmd(
        nc, in_maps, core_ids=list(range(N_CORES)), trace=trace,
        **(trace_kwargs or {}))
    out = np.empty((B, CIN, N), np.float32)
    for c in range(N_CORES):
        b, s = c // 2, c % 2
        out[b, :, s * NQ:(s + 1) * NQ] = res.results[c]["OUT"]
    return out.reshape(B, CIN, 14, 14, 14), res


def kernel(**inputs):
    out, _ = run(inputs, trace=False)
    return out
